# revision 28
# baseline (speedup 1.0000x reference)
"""Trainium2 8-core kernel for the GConvGRU-style GNN message-passing net.

Reference computation (N=100000 nodes, E=400000 edges, y = out[:50000]):
    deg  = indeg(dst) + 1;  dinv = rsqrt(deg)
    xs   = D^-1/2 (A + I) D^-1/2 x          # [N, 32] normalized aggregation
    cz   = xs @ Wz + bz ; ch = xs @ Wh + bh # (H == 0 for this problem)
    Z    = sigmoid(cz @ Lz_top + Lz_b); H~ = tanh(ch @ Lh_top + Lh_b)
    Hn   = (1 - Z) * H~
    y    = relu(Hn) @ W_out + b_out         # rows [0, 50000)

Only nodes < 50000 reach the output, so only their in-edges matter.

Sharding: 8 cores x 6250 output nodes. The host stages, per core, a
feature-major bf16 "slot stream" in DRAM — one column per (node,
sub-slot), fully pre-normalized (dinv[src]*dinv[dst]*x edge slots,
dinv^2*x self slot), a node's slots dealt round-robin over 4 k-groups
stacked 4x32 on the partition axis. The device does all arithmetic:

  - PE accumulates the slot sum directly from the stream into PSUM
    (per run of equal-depth chunks: k matmuls with start/stop
    accumulation), folding both the 4-group sum (via the 128-deep
    contraction against the 4x-tiled folded gate weights) and the
    sub-slot sum (via PSUM accumulate). No separate collapse pass.
  - ACT applies sigmoid/tanh per 1024-col superblock (PSUM -> SBUF).
  - DVE fuses relu+gating: prr = (ht max 0) * zc  [one STT op], then
    adds b_out while moving y out of PSUM (tensor_scalar_add).
  - Superblocks are processed smallest-stream-first so compute starts
    as soon as the first (smallest) DMA piece lands; stream pieces are
    issued back-to-back on the sync HWDGE queue and pipeline at line
    rate while the PE consumes earlier pieces.
"""
import os
import sys

import numpy as np

for _p in ("/root/.axon_site", "/root/.axon_site/_ro/trn_rl_repo",
           "/root/.axon_site/_ro/pypackages", "/opt/trn_rl_repo"):
    if os.path.isdir(_p) and _p not in sys.path:
        sys.path.append(_p)

N = 100000
E = 400000
DIN = 32
FLT = 128
NP_ = 8
NA = 50000
NCORES = 8
NODES_PER_CORE = NA // NCORES           # 6250
P = 128
NCHUNK = 49                             # chunks of 128 node cols
NCOL = NCHUNK * P                       # 6272 compute cols
SB_CHUNKS = 12                          # chunks per superblock (1536 cols)

_cache = {}


def _split_sync_waits(nc, mybir, limit=1):
    """walrus CoreV3 codegen supports one sync-wait per instruction."""
    cnt = 0
    for fn in nc.m.functions:
        for bb in fn.blocks:
            insts = list(bb.instructions)
            out = []
            changed = False
            for inst in insts:
                si = inst.sync_info
                if si is not None and si.on_wait is not None and len(si.on_wait) > limit:
                    w = list(si.on_wait)
                    upd = list(si.on_update) if si.on_update else []
                    chunks = [w[i:i + limit] for i in range(0, len(w), limit)]
                    for chunk in chunks[:-1]:
                        d = mybir.InstDrain(name=f"I-wsplit{cnt}", ins=[], outs=[])
                        cnt += 1
                        d.engine = inst.engine
                        d.sync_info = mybir.SyncInfo(on_wait=chunk, on_update=[])
                        out.append(d)
                    inst.sync_info = mybir.SyncInfo(on_wait=chunks[-1], on_update=upd)
                    changed = True
                out.append(inst)
            if changed:
                bb.instructions = out


def _plan(kq):
    """Static schedule shared by all cores.

    Superblocks of SB_CHUNKS chunks; within each 4-chunk half, runs of
    equal slot depth k (so every matmul's PSUM out stays inside one
    512-col bank). Superblocks are processed smallest-stream-first.
    Returns (sbs, order, CS) where sbs[s] = (chunk_lo, chunk_hi, runs,
    stream_off, stream_cols) with runs = [(chunk_lo, nchunks, k,
    stream_off_within_sb)], offsets assigned in process order.
    """
    kq = np.asarray(kq)
    bounds = list(range(0, NCHUNK, SB_CHUNKS)) + [NCHUNK]
    raw = []
    for lo, hi in zip(bounds[:-1], bounds[1:]):
        runs = []
        cols = 0
        for hlo in range(lo, hi, 4):
            hhi = min(hlo + 4, hi)
            c = hlo
            while c < hhi:
                k = int(kq[c])
                e = c
                while e < hhi and kq[e] == k:
                    e += 1
                runs.append((c, e - c, k, cols))
                cols += k * (e - c) * P
                c = e
        raw.append((lo, hi, runs, cols))
    # Process order: dense-stream superblocks first (keeps the PE busy so
    # HAM reaches full clock early; their pieces also stream longest), the
    # light k=1 superblocks last so the post-tanh drain is short. The
    # FIRST one is the second-densest so its (smaller) piece lands sooner.
    desc = sorted(range(len(raw)), key=lambda s: (-raw[s][3], s))
    order = desc[1:2] + desc[0:1] + desc[2:]
    sbs = []
    off = 0
    offs = {}
    for s in order:
        offs[s] = off
        off += raw[s][3]
    for s, (lo, hi, runs, cols) in enumerate(raw):
        sbs.append((lo, hi, runs, offs[s], cols))
    return sbs, order, off


def _build_device_kernel(kq):
    import concourse.bacc as bacc
    import concourse.mybir as mybir
    from concourse.tile import TileContext

    sbs, order, CS = _plan(kq)

    nc = bacc.Bacc("TRN2")
    f32 = mybir.dt.float32
    bf16 = mybir.dt.bfloat16

    tabS = nc.declare_dram_parameter("tabS", [P, CS], bf16, isOutput=False)
    # all constants in two DMAs: cb = azS | ahS | wout (bf16),
    # cf = -az | ah | b_out (f32) — six separate const DMAs serialized
    # ~6us on the scalar queue and stalled the first LDWEIGHTS.
    cb = nc.declare_dram_parameter("cb", [P, 2 * FLT + NP_], bf16, isOutput=False)
    cf = nc.declare_dram_parameter("cf", [P, 3], f32, isOutput=False)
    yout = nc.declare_dram_parameter("y", [NP_, NCOL], f32, isOutput=True)

    with TileContext(nc) as tc:
        with (
            tc.tile_pool(name="const", bufs=1) as cp,
            tc.tile_pool(name="st", bufs=1) as sp,
            tc.tile_pool(name="uzh", bufs=2, space="PSUM") as pz,
            tc.tile_pool(name="yp", bufs=2, space="PSUM") as yp,
            tc.tile_pool(name="zc", bufs=2) as zcp,
            tc.tile_pool(name="ht", bufs=2) as htp,
            tc.tile_pool(name="pr", bufs=2) as prp,
        ):
            # constants on the scalar HWDGE queue (stream uses sync's)
            cb_t = cp.tile([P, 2 * FLT + NP_], bf16)
            nc.scalar.dma_start(out=cb_t[:], in_=cb[:, :])
            cf_t = cp.tile([P, 3], f32)
            nc.scalar.dma_start(out=cf_t[:], in_=cf[:, :])
            azS_t = cb_t[:, 0:FLT]
            ahS_t = cb_t[:, FLT:2 * FLT]
            wout_t = cb_t[:, 2 * FLT:2 * FLT + NP_]
            azn_t = cf_t[:, 0:1]
            ahb_t = cf_t[:, 1:2]
            bout_t = cf_t[:, 2:3]

            # stream pieces, one per superblock, issued in process order
            st_tiles = {}
            for s in order:
                lo, hi, runs, soff, cols = sbs[s]
                st = sp.tile([P, cols], bf16, tag=f"st{s}")
                nc.sync.dma_start(out=st[:], in_=tabS[:, soff:soff + cols])
                st_tiles[s] = st

            y_sb = cp.tile([NP_, NCOL], f32)
            dum = cp.tile([FLT, 1], bf16)

            # preload both ACT function tables during the DMA head
            nc.scalar.activation(
                out=dum[:], in_=azn_t[:, :1],
                func=mybir.ActivationFunctionType.Tanh, bias=ahb_t[:, :1],
                scale=1.0)
            nc.scalar.activation(
                out=dum[:], in_=azn_t[:, :1],
                func=mybir.ActivationFunctionType.Sigmoid, bias=ahb_t[:, :1],
                scale=-1.0)

            for si, s in enumerate(order):
                lo, hi, runs, soff, cols = sbs[s]
                st = st_tiles[s]
                wsb = (hi - lo) * P
                sbcol0 = lo * P

                uz = pz.tile([P, wsb], f32, tag="uzh")
                uh = pz.tile([P, wsb], f32, tag="uzh")
                for lhsT, ups in ((azS_t, uz), (ahS_t, uh)):
                    for rlo, rn, rk, roff in runs:
                        w = rn * P
                        nod0 = (rlo - lo) * P
                        for j in range(rk):
                            nc.tensor.matmul(
                                out=ups[:, nod0:nod0 + w], lhsT=lhsT[:],
                                rhs=st[:, roff + j * w:roff + (j + 1) * w],
                                start=(j == 0), stop=(j == rk - 1))

                zc = zcp.tile([FLT, wsb], bf16, tag="zc")
                nc.scalar.activation(
                    out=zc[:], in_=uz[:],
                    func=mybir.ActivationFunctionType.Sigmoid,
                    bias=azn_t[:, :1], scale=-1.0)
                ht = htp.tile([FLT, wsb], bf16, tag="ht")
                nc.scalar.activation(
                    out=ht[:], in_=uh[:],
                    func=mybir.ActivationFunctionType.Tanh,
                    bias=ahb_t[:, :1], scale=1.0)

                # post-tanh chain at 512-block granularity so the drain of
                # the final superblocks pipelines across engines
                tail = si >= len(order) - 2
                prr = prp.tile([FLT, wsb], bf16, tag="pr")
                for h in range(0, wsb, 512):
                    w2 = min(512, wsb - h)
                    # fused relu+gating on DVE: prr = (ht max 0) * zc
                    nc.vector.scalar_tensor_tensor(
                        out=prr[:, h:h + w2], in0=ht[:, h:h + w2], scalar=0.0,
                        in1=zc[:, h:h + w2],
                        op0=mybir.AluOpType.max, op1=mybir.AluOpType.mult)
                    ypt = yp.tile([NP_, w2], f32, tag="yp")
                    nc.tensor.matmul(out=ypt[:], lhsT=wout_t[:],
                                     rhs=prr[:, h:h + w2],
                                     start=True, stop=True)
                    if tail and si < len(order) - 1:
                        # ACT is idle after its last tanh; DVE still owns
                        # the STTs — split the drain across both
                        nc.scalar.activation(
                            out=y_sb[:, sbcol0 + h:sbcol0 + h + w2],
                            in_=ypt[:],
                            func=mybir.ActivationFunctionType.Identity,
                            bias=bout_t[:NP_, :1], scale=1.0)
                    else:
                        nc.vector.tensor_scalar_add(
                            out=y_sb[:, sbcol0 + h:sbcol0 + h + w2], in0=ypt[:],
                            scalar1=bout_t[:NP_, :1])
                    if tail:
                        nc.sync.dma_start(
                            out=yout[:, sbcol0 + h:sbcol0 + h + w2],
                            in_=y_sb[:, sbcol0 + h:sbcol0 + h + w2])
                if not tail:
                    nc.gpsimd.dma_start(out=yout[:, sbcol0:sbcol0 + wsb],
                                        in_=y_sb[:, sbcol0:sbcol0 + wsb])

    import concourse.mybir as mybir2
    _split_sync_waits(nc, mybir2)
    nc.compile()
    return nc


def _numpy_fallback(x, H, edge_index, Wz, bz, Wr, br, Wh, bh,
                    Lz_w, Lz_b, Lr_w, Lr_b, Lh_w, Lh_b, W_out, b_out):
    """Exact replica of the reference for unexpected inputs (H != 0)."""
    src = np.asarray(edge_index[0], dtype=np.int64)
    dst = np.asarray(edge_index[1], dtype=np.int64)
    deg = np.zeros(N, np.float32)
    np.add.at(deg, dst, 1.0)
    deg += 1.0
    dinv = (1.0 / np.sqrt(deg)).astype(np.float32)

    def gcn(W, b):
        h = x @ W
        norm = (dinv[src] * dinv[dst]).astype(np.float32)
        agg = np.zeros_like(h)
        np.add.at(agg, dst, h[src] * norm[:, None])
        agg = agg + h * (dinv * dinv)[:, None]
        return agg + b

    def sigmoid(v):
        return 1.0 / (1.0 + np.exp(-v))

    cz = gcn(Wz, bz)
    cr = gcn(Wr, br)
    ch = gcn(Wh, bh)
    Z = sigmoid(np.concatenate([cz, H], axis=1) @ Lz_w + Lz_b)
    R = sigmoid(np.concatenate([cr, H], axis=1) @ Lr_w + Lr_b)
    Ht = np.tanh(np.concatenate([ch, H * R], axis=1) @ Lh_w + Lh_b)
    Hn = Z * H + (1.0 - Z) * Ht
    y = np.maximum(Hn, 0.0) @ W_out + b_out
    return y[:NA].astype(np.float32)


def kernel(x, H, edge_index, Wz, bz, Wr, br, Wh, bh,
           Lz_w, Lz_b, Lr_w, Lr_b, Lh_w, Lh_b, W_out, b_out):
    x = np.asarray(x, dtype=np.float32)
    H = np.asarray(H)
    if H.size and np.any(H):
        return _numpy_fallback(x, np.asarray(H, np.float32), edge_index,
                               np.asarray(Wz, np.float32), np.asarray(bz, np.float32),
                               np.asarray(Wr, np.float32), np.asarray(br, np.float32),
                               np.asarray(Wh, np.float32), np.asarray(bh, np.float32),
                               np.asarray(Lz_w, np.float32), np.asarray(Lz_b, np.float32),
                               np.asarray(Lr_w, np.float32), np.asarray(Lr_b, np.float32),
                               np.asarray(Lh_w, np.float32), np.asarray(Lh_b, np.float32),
                               np.asarray(W_out, np.float32), np.asarray(b_out, np.float32))

    import ml_dtypes
    bf = ml_dtypes.bfloat16

    src = np.asarray(edge_index[0], dtype=np.int64)
    dst = np.asarray(edge_index[1], dtype=np.int64)

    # --- normalization ---
    deg = np.bincount(dst, minlength=N).astype(np.float32) + 1.0
    dinv = (1.0 / np.sqrt(deg)).astype(np.float32)
    xs_pre = x * dinv[:, None]                                  # dinv[s] * x[s]

    # --- folded gate weights (H = 0 path) ---
    Wz = np.asarray(Wz, np.float32); Wh = np.asarray(Wh, np.float32)
    Lz_top = np.asarray(Lz_w, np.float32)[:FLT]
    Lh_top = np.asarray(Lh_w, np.float32)[:FLT]
    Az = Wz @ Lz_top                                            # [32,128]
    Ah = Wh @ Lh_top
    az = (np.asarray(bz, np.float32) @ Lz_top + np.asarray(Lz_b, np.float32)).astype(np.float32)
    ah = (np.asarray(bh, np.float32) @ Lh_top + np.asarray(Lh_b, np.float32)).astype(np.float32)
    Wout = np.asarray(W_out, np.float32).astype(bf)             # [128,8]
    bout = np.asarray(b_out, np.float32)                        # [8]
    # y-block h of a superblock lands at partitions 32h..32h+7
    bb128 = np.zeros((P, 1), np.float32)
    for h in range(4):
        bb128[32 * h:32 * h + NP_, 0] = bout

    # --- live edges: only dst < NA contribute to the output ---
    live = dst < NA
    srcL = src[live]
    dstL = dst[live]

    # per-core degree-sorted packing; uniform slot profile across cores
    per_core = []
    counts_sorted_all = np.zeros((NCORES, NCOL), np.int64)
    for c in range(NCORES):
        lo, hi = c * NODES_PER_CORE, (c + 1) * NODES_PER_CORE
        m = (dstL >= lo) & (dstL < hi)
        s_c = srcL[m]
        d_c = dstL[m] - lo
        cnt = np.bincount(d_c, minlength=NODES_PER_CORE)
        perm = np.argsort(-cnt, kind="stable")
        counts_sorted_all[c, :NODES_PER_CORE] = cnt[perm]
        per_core.append((s_c, d_c, cnt, perm))

    # per-chunk sub-slot depth (incl. self slot), shared by all cores
    kq = np.zeros(NCHUNK, np.int64)
    for ci in range(NCHUNK):
        kp = counts_sorted_all[:, ci * P:(ci + 1) * P].max() + 1
        kq[ci] = (kp + 3) // 4
    KMAX = int(kq.max())

    sbs, s_order, CS = _plan(kq)
    # base stream col of (chunk ci, sub-slot j), -1 = unused
    base_col = np.full((NCHUNK, KMAX), -1, np.int64)
    for lo, hi, runs, soff, cols in sbs:
        for rlo, rn, rk, roff in runs:
            w = rn * P
            for j in range(rk):
                for ci in range(rlo, rlo + rn):
                    base_col[ci, j] = soff + roff + j * w + (ci - rlo) * P

    in_maps = []
    perms = []
    azS = np.tile(Az, (4, 1)).astype(bf)                        # [128,128]
    ahS = np.tile(Ah, (4, 1)).astype(bf)
    r_all = np.arange(NODES_PER_CORE)
    ci_all = r_all // P
    p_all = r_all % P
    for c in range(NCORES):
        s_c, d_c, cnt, perm = per_core[c]
        invperm = np.empty(NODES_PER_CORE, np.int64)
        invperm[perm] = np.arange(NODES_PER_CORE)
        gids = perm + c * NODES_PER_CORE                        # rank -> node id

        # dinv[dst] per stream column (same for all k-groups)
        dvcol = np.zeros(CS, np.float32)
        for j in range(KMAX):
            mvalid = base_col[ci_all, j] >= 0
            rr = r_all[mvalid]
            dvcol[base_col[ci_all[mvalid], j] + p_all[mvalid]] = dinv[gids[rr]]

        # slot source table: [4 k-groups, CS cols], -1 = pad (zeros)
        slotsrc = np.full((4, CS), -1, np.int64)
        # self slots (k = 0 -> group 0, sub-slot 0)
        slotsrc[0, base_col[ci_all, 0] + p_all] = gids
        # edge slots (k = 1 + within-count)
        rk = invperm[d_c]
        eorder = np.argsort(rk, kind="stable")
        rk_s = rk[eorder]
        s_s = s_c[eorder]
        starts = np.zeros(NODES_PER_CORE + 1, np.int64)
        np.cumsum(cnt[perm], out=starts[1:])
        within = np.arange(len(rk_s)) - starts[rk_s]
        k = within + 1
        cole = base_col[rk_s // P, k // 4] + (rk_s % P)
        slotsrc[k % 4, cole] = s_s

        tabS = np.zeros((P, CS), bf)
        for g in range(4):
            vals = np.zeros((CS, DIN), np.float32)
            mm = slotsrc[g] >= 0
            vals[mm] = xs_pre[slotsrc[g][mm]]
            vals *= dvcol[:, None]
            tabS[32 * g:32 * g + 32, :] = vals.T.astype(bf)

        perms.append(perm)
        in_maps.append({
            "tabS": tabS,
            "cb": np.concatenate([azS, ahS, Wout], axis=1),
            "cf": np.stack([-az, ah, bb128[:, 0]], axis=1).astype(np.float32),
        })

    if os.environ.get("KERNEL_DEBUG") == "1":
        print(f"[kernel] kq={kq.tolist()} CS={CS} order={s_order} "
              f"stream={P * CS * 2 / 1e6:.2f}MB/core")
    key = ("v12", tuple(kq.tolist()))
    if key not in _cache:
        _cache[key] = _build_device_kernel(kq)
    nc = _cache[key]

    from concourse.bass_utils import run_bass_kernel_spmd
    trace = os.environ.get("KERNEL_TRACE") == "1"
    kwargs = {}
    if trace:
        kwargs = {"trace": True, "tmpdir": os.environ.get("KERNEL_TRACE_DIR", "/tmp/kernel_trace")}
    res = run_bass_kernel_spmd(nc, in_maps, list(range(NCORES)), **kwargs)
    global last_result
    last_result = res

    y = np.empty((NA, NP_), np.float32)
    for c in range(NCORES):
        yc = res.results[c]["y"]                                # [8, 6272]
        lo = c * NODES_PER_CORE
        y[lo + perms[c], :] = yc[:, :NODES_PER_CORE].T
    return y


# revision 32
# speedup vs baseline: 1.0543x; 1.0543x over previous
"""Trainium2 8-core kernel for the GConvGRU-style GNN message-passing net.

Reference computation (N=100000 nodes, E=400000 edges, y = out[:50000]):
    deg  = indeg(dst) + 1;  dinv = rsqrt(deg)
    xs   = D^-1/2 (A + I) D^-1/2 x          # [N, 32] normalized aggregation
    cz   = xs @ Wz + bz ; ch = xs @ Wh + bh # (H == 0 for this problem)
    Z    = sigmoid(cz @ Lz_top + Lz_b); H~ = tanh(ch @ Lh_top + Lh_b)
    Hn   = (1 - Z) * H~
    y    = relu(Hn) @ W_out + b_out         # rows [0, 50000)

Only nodes < 50000 reach the output, so only their in-edges matter.

Sharding: 8 cores x 6250 output nodes. The host stages, per core, a
feature-major bf16 "slot stream" in DRAM — one column per (node,
sub-slot), fully pre-normalized (dinv[src]*dinv[dst]*x edge slots,
dinv^2*x self slot), a node's slots dealt round-robin over 4 k-groups
stacked 4x32 on the partition axis. The device does all arithmetic:

  - PE accumulates the slot sum directly from the stream into PSUM
    (per run of equal-depth chunks: k matmuls with start/stop
    accumulation), folding both the 4-group sum (via the 128-deep
    contraction against the 4x-tiled folded gate weights) and the
    sub-slot sum (via PSUM accumulate). No separate collapse pass.
  - ACT applies sigmoid/tanh per 1024-col superblock (PSUM -> SBUF).
  - DVE fuses relu+gating: prr = (ht max 0) * zc  [one STT op], then
    adds b_out while moving y out of PSUM (tensor_scalar_add).
  - Superblocks are processed smallest-stream-first so compute starts
    as soon as the first (smallest) DMA piece lands; stream pieces are
    issued back-to-back on the sync HWDGE queue and pipeline at line
    rate while the PE consumes earlier pieces.
"""
import os
import sys

import numpy as np

for _p in ("/root/.axon_site", "/root/.axon_site/_ro/trn_rl_repo",
           "/root/.axon_site/_ro/pypackages", "/opt/trn_rl_repo"):
    if os.path.isdir(_p) and _p not in sys.path:
        sys.path.append(_p)

N = 100000
E = 400000
DIN = 32
FLT = 128
NP_ = 8
NA = 50000
NCORES = 8
NODES_PER_CORE = NA // NCORES           # 6250
P = 128
NCHUNK = 49                             # chunks of 128 node cols
NCOL = NCHUNK * P                       # 6272 compute cols
SB_CHUNKS = 12                          # chunks per superblock (1536 cols)

_cache = {}


def _split_sync_waits(nc, mybir, limit=1):
    """walrus CoreV3 codegen supports one sync-wait per instruction."""
    cnt = 0
    for fn in nc.m.functions:
        for bb in fn.blocks:
            insts = list(bb.instructions)
            out = []
            changed = False
            for inst in insts:
                si = inst.sync_info
                if si is not None and si.on_wait is not None and len(si.on_wait) > limit:
                    w = list(si.on_wait)
                    upd = list(si.on_update) if si.on_update else []
                    chunks = [w[i:i + limit] for i in range(0, len(w), limit)]
                    for chunk in chunks[:-1]:
                        d = mybir.InstDrain(name=f"I-wsplit{cnt}", ins=[], outs=[])
                        cnt += 1
                        d.engine = inst.engine
                        d.sync_info = mybir.SyncInfo(on_wait=chunk, on_update=[])
                        out.append(d)
                    inst.sync_info = mybir.SyncInfo(on_wait=chunks[-1], on_update=upd)
                    changed = True
                out.append(inst)
            if changed:
                bb.instructions = out


def _plan(kq):
    """Static schedule shared by all cores.

    Superblocks of SB_CHUNKS chunks; within each 4-chunk half, runs of
    equal slot depth k (so every matmul's PSUM out stays inside one
    512-col bank). Superblocks are processed smallest-stream-first.
    Returns (sbs, order, CS) where sbs[s] = (chunk_lo, chunk_hi, runs,
    stream_off, stream_cols) with runs = [(chunk_lo, nchunks, k,
    stream_off_within_sb)], offsets assigned in process order.
    """
    kq = np.asarray(kq)
    bounds = list(range(0, NCHUNK, SB_CHUNKS)) + [NCHUNK]
    raw = []
    for lo, hi in zip(bounds[:-1], bounds[1:]):
        runs = []
        cols = 0
        for hlo in range(lo, hi, 4):
            hhi = min(hlo + 4, hi)
            c = hlo
            while c < hhi:
                k = int(kq[c])
                e = c
                while e < hhi and kq[e] == k:
                    e += 1
                runs.append((c, e - c, k, cols))
                cols += k * (e - c) * P
                c = e
        raw.append((lo, hi, runs, cols))
    # Process order: start with a medium superblock (its piece lands
    # early but still feeds the PE densely), then the heavy ones, then
    # the light k=1 superblocks last so the post-tanh drain is short.
    desc = sorted(range(len(raw)), key=lambda s: (-raw[s][3], s))
    order = desc[2:3] + desc[0:2] + desc[3:]
    sbs = []
    off = 0
    offs = {}
    for s in order:
        offs[s] = off
        off += raw[s][3]
    for s, (lo, hi, runs, cols) in enumerate(raw):
        sbs.append((lo, hi, runs, offs[s], cols))
    return sbs, order, off


def _build_device_kernel(kq):
    import concourse.bacc as bacc
    import concourse.mybir as mybir
    from concourse.tile import TileContext

    sbs, order, CS = _plan(kq)

    nc = bacc.Bacc("TRN2")
    f32 = mybir.dt.float32
    bf16 = mybir.dt.bfloat16

    tabS = nc.declare_dram_parameter("tabS", [P, CS], bf16, isOutput=False)
    # all constants in two DMAs: cb = azS | ahS | wout (bf16),
    # cf = -az | ah | b_out (f32) — six separate const DMAs serialized
    # ~6us on the scalar queue and stalled the first LDWEIGHTS.
    cb = nc.declare_dram_parameter("cb", [P, 2 * FLT + NP_], bf16, isOutput=False)
    cf = nc.declare_dram_parameter("cf", [P, 3], f32, isOutput=False)
    yout = nc.declare_dram_parameter("y", [NP_, NCOL], f32, isOutput=True)

    with TileContext(nc) as tc:
        with (
            tc.tile_pool(name="const", bufs=1) as cp,
            tc.tile_pool(name="st", bufs=1) as sp,
            tc.tile_pool(name="uzh", bufs=2, space="PSUM") as pz,
            tc.tile_pool(name="yp", bufs=2, space="PSUM") as yp,
            tc.tile_pool(name="zc", bufs=2) as zcp,
            tc.tile_pool(name="ht", bufs=2) as htp,
            tc.tile_pool(name="pr", bufs=2) as prp,
        ):
            # constants FIRST on the sync queue: FIFO puts them ahead of
            # the big stream pieces (on the scalar ring they finished
            # ~8.5us in — the SDMA engines round-robin rings at packet
            # granularity, so small transfers drown behind big ones)
            cb_t = cp.tile([P, 2 * FLT + NP_], bf16)
            nc.sync.dma_start(out=cb_t[:], in_=cb[:, :])
            cf_t = cp.tile([P, 3], f32)
            nc.sync.dma_start(out=cf_t[:], in_=cf[:, :])
            azS_t = cb_t[:, 0:FLT]
            ahS_t = cb_t[:, FLT:2 * FLT]
            wout_t = cb_t[:, 2 * FLT:2 * FLT + NP_]
            azn_t = cf_t[:, 0:1]
            ahb_t = cf_t[:, 1:2]
            bout_t = cf_t[:, 2:3]

            # stream pieces, one per superblock, issued in process order
            st_tiles = {}
            for s in order:
                lo, hi, runs, soff, cols = sbs[s]
                st = sp.tile([P, cols], bf16, tag=f"st{s}")
                nc.sync.dma_start(out=st[:], in_=tabS[:, soff:soff + cols])
                st_tiles[s] = st

            y_sb = cp.tile([NP_, NCOL], f32)
            dum = cp.tile([FLT, 1], bf16)

            # preload both ACT function tables during the DMA head
            nc.scalar.activation(
                out=dum[:], in_=azn_t[:, :1],
                func=mybir.ActivationFunctionType.Tanh, bias=ahb_t[:, :1],
                scale=1.0)
            nc.scalar.activation(
                out=dum[:], in_=azn_t[:, :1],
                func=mybir.ActivationFunctionType.Sigmoid, bias=ahb_t[:, :1],
                scale=-1.0)

            for si, s in enumerate(order):
                lo, hi, runs, soff, cols = sbs[s]
                st = st_tiles[s]
                wsb = (hi - lo) * P
                sbcol0 = lo * P

                uz = pz.tile([P, wsb], f32, tag="uzh")
                uh = pz.tile([P, wsb], f32, tag="uzh")
                for lhsT, ups in ((azS_t, uz), (ahS_t, uh)):
                    for rlo, rn, rk, roff in runs:
                        w = rn * P
                        nod0 = (rlo - lo) * P
                        for j in range(rk):
                            nc.tensor.matmul(
                                out=ups[:, nod0:nod0 + w], lhsT=lhsT[:],
                                rhs=st[:, roff + j * w:roff + (j + 1) * w],
                                start=(j == 0), stop=(j == rk - 1))

                zc = zcp.tile([FLT, wsb], bf16, tag="zc")
                nc.scalar.activation(
                    out=zc[:], in_=uz[:],
                    func=mybir.ActivationFunctionType.Sigmoid,
                    bias=azn_t[:, :1], scale=-1.0)
                ht = htp.tile([FLT, wsb], bf16, tag="ht")
                nc.scalar.activation(
                    out=ht[:], in_=uh[:],
                    func=mybir.ActivationFunctionType.Tanh,
                    bias=ahb_t[:, :1], scale=1.0)

                # post-tanh chain at 512-block granularity so the drain of
                # the final superblocks pipelines across engines
                tail = si >= len(order) - 2
                prr = prp.tile([FLT, wsb], bf16, tag="pr")
                for h in range(0, wsb, 512):
                    w2 = min(512, wsb - h)
                    # fused relu+gating on DVE: prr = (ht max 0) * zc
                    nc.vector.scalar_tensor_tensor(
                        out=prr[:, h:h + w2], in0=ht[:, h:h + w2], scalar=0.0,
                        in1=zc[:, h:h + w2],
                        op0=mybir.AluOpType.max, op1=mybir.AluOpType.mult)
                    ypt = yp.tile([NP_, w2], f32, tag="yp")
                    nc.tensor.matmul(out=ypt[:], lhsT=wout_t[:],
                                     rhs=prr[:, h:h + w2],
                                     start=True, stop=True)
                    if tail or (si == len(order) - 3 and h == 1024):
                        # ACT is idle after its last tanh; DVE still owns
                        # the STTs — split the drain across both
                        nc.scalar.activation(
                            out=y_sb[:, sbcol0 + h:sbcol0 + h + w2],
                            in_=ypt[:],
                            func=mybir.ActivationFunctionType.Identity,
                            bias=bout_t[:NP_, :1], scale=1.0)
                    else:
                        nc.vector.tensor_scalar_add(
                            out=y_sb[:, sbcol0 + h:sbcol0 + h + w2], in0=ypt[:],
                            scalar1=bout_t[:NP_, :1])
                    if tail:
                        nc.sync.dma_start(
                            out=yout[:, sbcol0 + h:sbcol0 + h + w2],
                            in_=y_sb[:, sbcol0 + h:sbcol0 + h + w2])
                if not tail:
                    nc.gpsimd.dma_start(out=yout[:, sbcol0:sbcol0 + wsb],
                                        in_=y_sb[:, sbcol0:sbcol0 + wsb])

    import concourse.mybir as mybir2
    _split_sync_waits(nc, mybir2)
    nc.compile()
    return nc


def _numpy_fallback(x, H, edge_index, Wz, bz, Wr, br, Wh, bh,
                    Lz_w, Lz_b, Lr_w, Lr_b, Lh_w, Lh_b, W_out, b_out):
    """Exact replica of the reference for unexpected inputs (H != 0)."""
    src = np.asarray(edge_index[0], dtype=np.int64)
    dst = np.asarray(edge_index[1], dtype=np.int64)
    deg = np.zeros(N, np.float32)
    np.add.at(deg, dst, 1.0)
    deg += 1.0
    dinv = (1.0 / np.sqrt(deg)).astype(np.float32)

    def gcn(W, b):
        h = x @ W
        norm = (dinv[src] * dinv[dst]).astype(np.float32)
        agg = np.zeros_like(h)
        np.add.at(agg, dst, h[src] * norm[:, None])
        agg = agg + h * (dinv * dinv)[:, None]
        return agg + b

    def sigmoid(v):
        return 1.0 / (1.0 + np.exp(-v))

    cz = gcn(Wz, bz)
    cr = gcn(Wr, br)
    ch = gcn(Wh, bh)
    Z = sigmoid(np.concatenate([cz, H], axis=1) @ Lz_w + Lz_b)
    R = sigmoid(np.concatenate([cr, H], axis=1) @ Lr_w + Lr_b)
    Ht = np.tanh(np.concatenate([ch, H * R], axis=1) @ Lh_w + Lh_b)
    Hn = Z * H + (1.0 - Z) * Ht
    y = np.maximum(Hn, 0.0) @ W_out + b_out
    return y[:NA].astype(np.float32)


def kernel(x, H, edge_index, Wz, bz, Wr, br, Wh, bh,
           Lz_w, Lz_b, Lr_w, Lr_b, Lh_w, Lh_b, W_out, b_out):
    x = np.asarray(x, dtype=np.float32)
    H = np.asarray(H)
    if H.size and np.any(H):
        return _numpy_fallback(x, np.asarray(H, np.float32), edge_index,
                               np.asarray(Wz, np.float32), np.asarray(bz, np.float32),
                               np.asarray(Wr, np.float32), np.asarray(br, np.float32),
                               np.asarray(Wh, np.float32), np.asarray(bh, np.float32),
                               np.asarray(Lz_w, np.float32), np.asarray(Lz_b, np.float32),
                               np.asarray(Lr_w, np.float32), np.asarray(Lr_b, np.float32),
                               np.asarray(Lh_w, np.float32), np.asarray(Lh_b, np.float32),
                               np.asarray(W_out, np.float32), np.asarray(b_out, np.float32))

    import ml_dtypes
    bf = ml_dtypes.bfloat16

    src = np.asarray(edge_index[0], dtype=np.int64)
    dst = np.asarray(edge_index[1], dtype=np.int64)

    # --- normalization ---
    deg = np.bincount(dst, minlength=N).astype(np.float32) + 1.0
    dinv = (1.0 / np.sqrt(deg)).astype(np.float32)
    xs_pre = x * dinv[:, None]                                  # dinv[s] * x[s]

    # --- folded gate weights (H = 0 path) ---
    Wz = np.asarray(Wz, np.float32); Wh = np.asarray(Wh, np.float32)
    Lz_top = np.asarray(Lz_w, np.float32)[:FLT]
    Lh_top = np.asarray(Lh_w, np.float32)[:FLT]
    Az = Wz @ Lz_top                                            # [32,128]
    Ah = Wh @ Lh_top
    az = (np.asarray(bz, np.float32) @ Lz_top + np.asarray(Lz_b, np.float32)).astype(np.float32)
    ah = (np.asarray(bh, np.float32) @ Lh_top + np.asarray(Lh_b, np.float32)).astype(np.float32)
    Wout = np.asarray(W_out, np.float32).astype(bf)             # [128,8]
    bout = np.asarray(b_out, np.float32)                        # [8]
    # y-block h of a superblock lands at partitions 32h..32h+7
    bb128 = np.zeros((P, 1), np.float32)
    for h in range(4):
        bb128[32 * h:32 * h + NP_, 0] = bout

    # --- live edges: only dst < NA contribute to the output ---
    live = dst < NA
    srcL = src[live]
    dstL = dst[live]

    # per-core degree-sorted packing; uniform slot profile across cores
    per_core = []
    counts_sorted_all = np.zeros((NCORES, NCOL), np.int64)
    for c in range(NCORES):
        lo, hi = c * NODES_PER_CORE, (c + 1) * NODES_PER_CORE
        m = (dstL >= lo) & (dstL < hi)
        s_c = srcL[m]
        d_c = dstL[m] - lo
        cnt = np.bincount(d_c, minlength=NODES_PER_CORE)
        perm = np.argsort(-cnt, kind="stable")
        counts_sorted_all[c, :NODES_PER_CORE] = cnt[perm]
        per_core.append((s_c, d_c, cnt, perm))

    # per-chunk sub-slot depth (incl. self slot), shared by all cores
    kq = np.zeros(NCHUNK, np.int64)
    for ci in range(NCHUNK):
        kp = counts_sorted_all[:, ci * P:(ci + 1) * P].max() + 1
        kq[ci] = (kp + 3) // 4
    KMAX = int(kq.max())

    sbs, s_order, CS = _plan(kq)
    # base stream col of (chunk ci, sub-slot j), -1 = unused
    base_col = np.full((NCHUNK, KMAX), -1, np.int64)
    for lo, hi, runs, soff, cols in sbs:
        for rlo, rn, rk, roff in runs:
            w = rn * P
            for j in range(rk):
                for ci in range(rlo, rlo + rn):
                    base_col[ci, j] = soff + roff + j * w + (ci - rlo) * P

    in_maps = []
    perms = []
    azS = np.tile(Az, (4, 1)).astype(bf)                        # [128,128]
    ahS = np.tile(Ah, (4, 1)).astype(bf)
    r_all = np.arange(NODES_PER_CORE)
    ci_all = r_all // P
    p_all = r_all % P
    for c in range(NCORES):
        s_c, d_c, cnt, perm = per_core[c]
        invperm = np.empty(NODES_PER_CORE, np.int64)
        invperm[perm] = np.arange(NODES_PER_CORE)
        gids = perm + c * NODES_PER_CORE                        # rank -> node id

        # dinv[dst] per stream column (same for all k-groups)
        dvcol = np.zeros(CS, np.float32)
        for j in range(KMAX):
            mvalid = base_col[ci_all, j] >= 0
            rr = r_all[mvalid]
            dvcol[base_col[ci_all[mvalid], j] + p_all[mvalid]] = dinv[gids[rr]]

        # slot source table: [4 k-groups, CS cols], -1 = pad (zeros)
        slotsrc = np.full((4, CS), -1, np.int64)
        # self slots (k = 0 -> group 0, sub-slot 0)
        slotsrc[0, base_col[ci_all, 0] + p_all] = gids
        # edge slots (k = 1 + within-count)
        rk = invperm[d_c]
        eorder = np.argsort(rk, kind="stable")
        rk_s = rk[eorder]
        s_s = s_c[eorder]
        starts = np.zeros(NODES_PER_CORE + 1, np.int64)
        np.cumsum(cnt[perm], out=starts[1:])
        within = np.arange(len(rk_s)) - starts[rk_s]
        k = within + 1
        cole = base_col[rk_s // P, k // 4] + (rk_s % P)
        slotsrc[k % 4, cole] = s_s

        tabS = np.zeros((P, CS), bf)
        for g in range(4):
            vals = np.zeros((CS, DIN), np.float32)
            mm = slotsrc[g] >= 0
            vals[mm] = xs_pre[slotsrc[g][mm]]
            vals *= dvcol[:, None]
            tabS[32 * g:32 * g + 32, :] = vals.T.astype(bf)

        perms.append(perm)
        in_maps.append({
            "tabS": tabS,
            "cb": np.concatenate([azS, ahS, Wout], axis=1),
            "cf": np.stack([-az, ah, bb128[:, 0]], axis=1).astype(np.float32),
        })

    if os.environ.get("KERNEL_DEBUG") == "1":
        print(f"[kernel] kq={kq.tolist()} CS={CS} order={s_order} "
              f"stream={P * CS * 2 / 1e6:.2f}MB/core")
    key = ("v13", tuple(kq.tolist()))
    if key not in _cache:
        _cache[key] = _build_device_kernel(kq)
    nc = _cache[key]

    from concourse.bass_utils import run_bass_kernel_spmd
    trace = os.environ.get("KERNEL_TRACE") == "1"
    kwargs = {}
    if trace:
        kwargs = {"trace": True, "tmpdir": os.environ.get("KERNEL_TRACE_DIR", "/tmp/kernel_trace")}
    res = run_bass_kernel_spmd(nc, in_maps, list(range(NCORES)), **kwargs)
    global last_result
    last_result = res

    y = np.empty((NA, NP_), np.float32)
    for c in range(NCORES):
        yc = res.results[c]["y"]                                # [8, 6272]
        lo = c * NODES_PER_CORE
        y[lo + perms[c], :] = yc[:, :NODES_PER_CORE].T
    return y


# revision 34
# speedup vs baseline: 1.1038x; 1.0470x over previous
"""Trainium2 8-core kernel for the GConvGRU-style GNN message-passing net.

Reference computation (N=100000 nodes, E=400000 edges, y = out[:50000]):
    deg  = indeg(dst) + 1;  dinv = rsqrt(deg)
    xs   = D^-1/2 (A + I) D^-1/2 x          # [N, 32] normalized aggregation
    cz   = xs @ Wz + bz ; ch = xs @ Wh + bh # (H == 0 for this problem)
    Z    = sigmoid(cz @ Lz_top + Lz_b); H~ = tanh(ch @ Lh_top + Lh_b)
    Hn   = (1 - Z) * H~
    y    = relu(Hn) @ W_out + b_out         # rows [0, 50000)

Only nodes < 50000 reach the output, so only their in-edges matter.

Sharding: 8 cores x 6250 output nodes. The host stages, per core, a
feature-major bf16 "slot stream" in DRAM — one column per (node,
sub-slot), fully pre-normalized (dinv[src]*dinv[dst]*x edge slots,
dinv^2*x self slot), a node's slots dealt round-robin over 4 k-groups
stacked 4x32 on the partition axis. The device does all arithmetic:

  - PE accumulates the slot sum directly from the stream into PSUM
    (per run of equal-depth chunks: k matmuls with start/stop
    accumulation), folding both the 4-group sum (via the 128-deep
    contraction against the 4x-tiled folded gate weights) and the
    sub-slot sum (via PSUM accumulate). No separate collapse pass.
  - ACT applies sigmoid/tanh per 1024-col superblock (PSUM -> SBUF).
  - DVE fuses relu+gating: prr = (ht max 0) * zc  [one STT op], then
    adds b_out while moving y out of PSUM (tensor_scalar_add).
  - Superblocks are processed smallest-stream-first so compute starts
    as soon as the first (smallest) DMA piece lands; stream pieces are
    issued back-to-back on the sync HWDGE queue and pipeline at line
    rate while the PE consumes earlier pieces.
"""
import os
import sys

import numpy as np

for _p in ("/root/.axon_site", "/root/.axon_site/_ro/trn_rl_repo",
           "/root/.axon_site/_ro/pypackages", "/opt/trn_rl_repo"):
    if os.path.isdir(_p) and _p not in sys.path:
        sys.path.append(_p)

N = 100000
E = 400000
DIN = 32
FLT = 128
NP_ = 8
NA = 50000
NCORES = 8
NODES_PER_CORE = NA // NCORES           # 6250
P = 128
NCHUNK = 49                             # chunks of 128 node cols
NCOL = NCHUNK * P                       # 6272 compute cols
SB_CHUNKS = 12                          # chunks per superblock (1536 cols)

_cache = {}


def _split_sync_waits(nc, mybir, limit=1):
    """walrus CoreV3 codegen supports one sync-wait per instruction."""
    cnt = 0
    for fn in nc.m.functions:
        for bb in fn.blocks:
            insts = list(bb.instructions)
            out = []
            changed = False
            for inst in insts:
                si = inst.sync_info
                if si is not None and si.on_wait is not None and len(si.on_wait) > limit:
                    w = list(si.on_wait)
                    upd = list(si.on_update) if si.on_update else []
                    chunks = [w[i:i + limit] for i in range(0, len(w), limit)]
                    for chunk in chunks[:-1]:
                        d = mybir.InstDrain(name=f"I-wsplit{cnt}", ins=[], outs=[])
                        cnt += 1
                        d.engine = inst.engine
                        d.sync_info = mybir.SyncInfo(on_wait=chunk, on_update=[])
                        out.append(d)
                    inst.sync_info = mybir.SyncInfo(on_wait=chunks[-1], on_update=upd)
                    changed = True
                out.append(inst)
            if changed:
                bb.instructions = out


def _plan(kq):
    """Static schedule shared by all cores.

    Superblocks of SB_CHUNKS chunks; within each 4-chunk half, runs of
    equal slot depth k (so every matmul's PSUM out stays inside one
    512-col bank). Superblocks are processed smallest-stream-first.
    Returns (sbs, order, CS) where sbs[s] = (chunk_lo, chunk_hi, runs,
    stream_off, stream_cols) with runs = [(chunk_lo, nchunks, k,
    stream_off_within_sb)], offsets assigned in process order.
    """
    kq = np.asarray(kq)
    bounds = list(range(0, NCHUNK, SB_CHUNKS)) + [NCHUNK]
    raw = []
    for lo, hi in zip(bounds[:-1], bounds[1:]):
        runs = []
        cols = 0
        for hlo in range(lo, hi, 4):
            hhi = min(hlo + 4, hi)
            c = hlo
            while c < hhi:
                k = int(kq[c])
                e = c
                while e < hhi and kq[e] == k:
                    e += 1
                runs.append((c, e - c, k, cols))
                cols += k * (e - c) * P
                c = e
        raw.append((lo, hi, runs, cols))
    # Process order: start with a medium superblock (its piece lands
    # early but still feeds the PE densely), then the heavy ones, then
    # the light k=1 superblocks last so the post-tanh drain is short.
    desc = sorted(range(len(raw)), key=lambda s: (-raw[s][3], s))
    order = desc[2:3] + desc[0:2] + desc[3:]
    sbs = []
    off = 0
    offs = {}
    for s in order:
        offs[s] = off
        off += raw[s][3]
    for s, (lo, hi, runs, cols) in enumerate(raw):
        sbs.append((lo, hi, runs, offs[s], cols))
    return sbs, order, off


def _build_device_kernel(kq):
    import concourse.bacc as bacc
    import concourse.mybir as mybir
    from concourse.tile import TileContext

    sbs, order, CS = _plan(kq)

    nc = bacc.Bacc("TRN2")
    f32 = mybir.dt.float32
    bf16 = mybir.dt.bfloat16

    tabS = nc.declare_dram_parameter("tabS", [P, CS], bf16, isOutput=False)
    # all constants in two DMAs: cb = azS | ahS | wout (bf16),
    # cf = -az | ah | b_out (f32) — six separate const DMAs serialized
    # ~6us on the scalar queue and stalled the first LDWEIGHTS.
    cb = nc.declare_dram_parameter("cb", [P, 2 * FLT + NP_], bf16, isOutput=False)
    cf = nc.declare_dram_parameter("cf", [P, 3], f32, isOutput=False)
    yout = nc.declare_dram_parameter("y", [NP_, NCOL], f32, isOutput=True)

    with TileContext(nc) as tc:
        with (
            tc.tile_pool(name="const", bufs=1) as cp,
            tc.tile_pool(name="st", bufs=1) as sp,
            tc.tile_pool(name="uzh", bufs=2, space="PSUM") as pz,
            tc.tile_pool(name="yp", bufs=2, space="PSUM") as yp,
            tc.tile_pool(name="zc", bufs=2) as zcp,
            tc.tile_pool(name="ht", bufs=2) as htp,
            tc.tile_pool(name="pr", bufs=2) as prp,
        ):
            # constants FIRST on the sync queue: FIFO puts them ahead of
            # the big stream pieces (on the scalar ring they finished
            # ~8.5us in — the SDMA engines round-robin rings at packet
            # granularity, so small transfers drown behind big ones)
            cb_t = cp.tile([P, 2 * FLT + NP_], bf16)
            nc.sync.dma_start(out=cb_t[:], in_=cb[:, :])
            cf_t = cp.tile([P, 3], f32)
            nc.sync.dma_start(out=cf_t[:], in_=cf[:, :])
            azS_t = cb_t[:, 0:FLT]
            ahS_t = cb_t[:, FLT:2 * FLT]
            wout_t = cb_t[:, 2 * FLT:2 * FLT + NP_]
            azn_t = cf_t[:, 0:1]
            ahb_t = cf_t[:, 1:2]
            bout_t = cf_t[:, 2:3]

            # stream pieces, one per superblock, issued in process order
            st_tiles = {}
            for s in order:
                lo, hi, runs, soff, cols = sbs[s]
                st = sp.tile([P, cols], bf16, tag=f"st{s}")
                nc.sync.dma_start(out=st[:], in_=tabS[:, soff:soff + cols])
                st_tiles[s] = st

            y_sb = cp.tile([NP_, NCOL], f32)
            dum = cp.tile([FLT, 1], bf16)

            # preload both ACT function tables during the DMA head
            nc.scalar.activation(
                out=dum[:], in_=azn_t[:, :1],
                func=mybir.ActivationFunctionType.Tanh, bias=ahb_t[:, :1],
                scale=1.0)
            nc.scalar.activation(
                out=dum[:], in_=azn_t[:, :1],
                func=mybir.ActivationFunctionType.Sigmoid, bias=ahb_t[:, :1],
                scale=-1.0)

            # PE warmup: ~3.4us of continuous matmuls (on const garbage,
            # into discarded y-pool tiles) flips HAM to 2.4 GHz before the
            # heavy gate matmuls arrive; without it they all run at 1.2.
            for _w in range(16):
                wt = yp.tile([NP_, 256], f32, tag="yp")
                nc.tensor.matmul(out=wt[:], lhsT=wout_t[:],
                                 rhs=cb_t[:, 0:256], start=True, stop=True)

            for si, s in enumerate(order):
                lo, hi, runs, soff, cols = sbs[s]
                st = st_tiles[s]
                wsb = (hi - lo) * P
                sbcol0 = lo * P

                uz = pz.tile([P, wsb], f32, tag="uzh")
                uh = pz.tile([P, wsb], f32, tag="uzh")
                for lhsT, ups in ((azS_t, uz), (ahS_t, uh)):
                    for rlo, rn, rk, roff in runs:
                        w = rn * P
                        nod0 = (rlo - lo) * P
                        for j in range(rk):
                            nc.tensor.matmul(
                                out=ups[:, nod0:nod0 + w], lhsT=lhsT[:],
                                rhs=st[:, roff + j * w:roff + (j + 1) * w],
                                start=(j == 0), stop=(j == rk - 1))

                zc = zcp.tile([FLT, wsb], bf16, tag="zc")
                nc.scalar.activation(
                    out=zc[:], in_=uz[:],
                    func=mybir.ActivationFunctionType.Sigmoid,
                    bias=azn_t[:, :1], scale=-1.0)
                ht = htp.tile([FLT, wsb], bf16, tag="ht")
                nc.scalar.activation(
                    out=ht[:], in_=uh[:],
                    func=mybir.ActivationFunctionType.Tanh,
                    bias=ahb_t[:, :1], scale=1.0)

                # post-tanh chain at 512-block granularity so the drain of
                # the final superblocks pipelines across engines
                tail = si >= len(order) - 2
                prr = prp.tile([FLT, wsb], bf16, tag="pr")
                for h in range(0, wsb, 512):
                    w2 = min(512, wsb - h)
                    # fused relu+gating on DVE: prr = (ht max 0) * zc
                    nc.vector.scalar_tensor_tensor(
                        out=prr[:, h:h + w2], in0=ht[:, h:h + w2], scalar=0.0,
                        in1=zc[:, h:h + w2],
                        op0=mybir.AluOpType.max, op1=mybir.AluOpType.mult)
                    ypt = yp.tile([NP_, w2], f32, tag="yp")
                    nc.tensor.matmul(out=ypt[:], lhsT=wout_t[:],
                                     rhs=prr[:, h:h + w2],
                                     start=True, stop=True)
                    if tail or (si == len(order) - 3 and h == 1024):
                        # ACT is idle after its last tanh; DVE still owns
                        # the STTs — split the drain across both
                        nc.scalar.activation(
                            out=y_sb[:, sbcol0 + h:sbcol0 + h + w2],
                            in_=ypt[:],
                            func=mybir.ActivationFunctionType.Identity,
                            bias=bout_t[:NP_, :1], scale=1.0)
                    else:
                        nc.vector.tensor_scalar_add(
                            out=y_sb[:, sbcol0 + h:sbcol0 + h + w2], in0=ypt[:],
                            scalar1=bout_t[:NP_, :1])
                    if tail:
                        nc.sync.dma_start(
                            out=yout[:, sbcol0 + h:sbcol0 + h + w2],
                            in_=y_sb[:, sbcol0 + h:sbcol0 + h + w2])
                if not tail:
                    nc.gpsimd.dma_start(out=yout[:, sbcol0:sbcol0 + wsb],
                                        in_=y_sb[:, sbcol0:sbcol0 + wsb])

    import concourse.mybir as mybir2
    _split_sync_waits(nc, mybir2)
    nc.compile()
    return nc


def _numpy_fallback(x, H, edge_index, Wz, bz, Wr, br, Wh, bh,
                    Lz_w, Lz_b, Lr_w, Lr_b, Lh_w, Lh_b, W_out, b_out):
    """Exact replica of the reference for unexpected inputs (H != 0)."""
    src = np.asarray(edge_index[0], dtype=np.int64)
    dst = np.asarray(edge_index[1], dtype=np.int64)
    deg = np.zeros(N, np.float32)
    np.add.at(deg, dst, 1.0)
    deg += 1.0
    dinv = (1.0 / np.sqrt(deg)).astype(np.float32)

    def gcn(W, b):
        h = x @ W
        norm = (dinv[src] * dinv[dst]).astype(np.float32)
        agg = np.zeros_like(h)
        np.add.at(agg, dst, h[src] * norm[:, None])
        agg = agg + h * (dinv * dinv)[:, None]
        return agg + b

    def sigmoid(v):
        return 1.0 / (1.0 + np.exp(-v))

    cz = gcn(Wz, bz)
    cr = gcn(Wr, br)
    ch = gcn(Wh, bh)
    Z = sigmoid(np.concatenate([cz, H], axis=1) @ Lz_w + Lz_b)
    R = sigmoid(np.concatenate([cr, H], axis=1) @ Lr_w + Lr_b)
    Ht = np.tanh(np.concatenate([ch, H * R], axis=1) @ Lh_w + Lh_b)
    Hn = Z * H + (1.0 - Z) * Ht
    y = np.maximum(Hn, 0.0) @ W_out + b_out
    return y[:NA].astype(np.float32)


def kernel(x, H, edge_index, Wz, bz, Wr, br, Wh, bh,
           Lz_w, Lz_b, Lr_w, Lr_b, Lh_w, Lh_b, W_out, b_out):
    x = np.asarray(x, dtype=np.float32)
    H = np.asarray(H)
    if H.size and np.any(H):
        return _numpy_fallback(x, np.asarray(H, np.float32), edge_index,
                               np.asarray(Wz, np.float32), np.asarray(bz, np.float32),
                               np.asarray(Wr, np.float32), np.asarray(br, np.float32),
                               np.asarray(Wh, np.float32), np.asarray(bh, np.float32),
                               np.asarray(Lz_w, np.float32), np.asarray(Lz_b, np.float32),
                               np.asarray(Lr_w, np.float32), np.asarray(Lr_b, np.float32),
                               np.asarray(Lh_w, np.float32), np.asarray(Lh_b, np.float32),
                               np.asarray(W_out, np.float32), np.asarray(b_out, np.float32))

    import ml_dtypes
    bf = ml_dtypes.bfloat16

    src = np.asarray(edge_index[0], dtype=np.int64)
    dst = np.asarray(edge_index[1], dtype=np.int64)

    # --- normalization ---
    deg = np.bincount(dst, minlength=N).astype(np.float32) + 1.0
    dinv = (1.0 / np.sqrt(deg)).astype(np.float32)
    xs_pre = x * dinv[:, None]                                  # dinv[s] * x[s]

    # --- folded gate weights (H = 0 path) ---
    Wz = np.asarray(Wz, np.float32); Wh = np.asarray(Wh, np.float32)
    Lz_top = np.asarray(Lz_w, np.float32)[:FLT]
    Lh_top = np.asarray(Lh_w, np.float32)[:FLT]
    Az = Wz @ Lz_top                                            # [32,128]
    Ah = Wh @ Lh_top
    az = (np.asarray(bz, np.float32) @ Lz_top + np.asarray(Lz_b, np.float32)).astype(np.float32)
    ah = (np.asarray(bh, np.float32) @ Lh_top + np.asarray(Lh_b, np.float32)).astype(np.float32)
    Wout = np.asarray(W_out, np.float32).astype(bf)             # [128,8]
    bout = np.asarray(b_out, np.float32)                        # [8]
    # y-block h of a superblock lands at partitions 32h..32h+7
    bb128 = np.zeros((P, 1), np.float32)
    for h in range(4):
        bb128[32 * h:32 * h + NP_, 0] = bout

    # --- live edges: only dst < NA contribute to the output ---
    live = dst < NA
    srcL = src[live]
    dstL = dst[live]

    # per-core degree-sorted packing; uniform slot profile across cores
    per_core = []
    counts_sorted_all = np.zeros((NCORES, NCOL), np.int64)
    for c in range(NCORES):
        lo, hi = c * NODES_PER_CORE, (c + 1) * NODES_PER_CORE
        m = (dstL >= lo) & (dstL < hi)
        s_c = srcL[m]
        d_c = dstL[m] - lo
        cnt = np.bincount(d_c, minlength=NODES_PER_CORE)
        perm = np.argsort(-cnt, kind="stable")
        counts_sorted_all[c, :NODES_PER_CORE] = cnt[perm]
        per_core.append((s_c, d_c, cnt, perm))

    # per-chunk sub-slot depth (incl. self slot), shared by all cores
    kq = np.zeros(NCHUNK, np.int64)
    for ci in range(NCHUNK):
        kp = counts_sorted_all[:, ci * P:(ci + 1) * P].max() + 1
        kq[ci] = (kp + 3) // 4
    KMAX = int(kq.max())

    sbs, s_order, CS = _plan(kq)
    # base stream col of (chunk ci, sub-slot j), -1 = unused
    base_col = np.full((NCHUNK, KMAX), -1, np.int64)
    for lo, hi, runs, soff, cols in sbs:
        for rlo, rn, rk, roff in runs:
            w = rn * P
            for j in range(rk):
                for ci in range(rlo, rlo + rn):
                    base_col[ci, j] = soff + roff + j * w + (ci - rlo) * P

    in_maps = []
    perms = []
    azS = np.tile(Az, (4, 1)).astype(bf)                        # [128,128]
    ahS = np.tile(Ah, (4, 1)).astype(bf)
    r_all = np.arange(NODES_PER_CORE)
    ci_all = r_all // P
    p_all = r_all % P
    for c in range(NCORES):
        s_c, d_c, cnt, perm = per_core[c]
        invperm = np.empty(NODES_PER_CORE, np.int64)
        invperm[perm] = np.arange(NODES_PER_CORE)
        gids = perm + c * NODES_PER_CORE                        # rank -> node id

        # dinv[dst] per stream column (same for all k-groups)
        dvcol = np.zeros(CS, np.float32)
        for j in range(KMAX):
            mvalid = base_col[ci_all, j] >= 0
            rr = r_all[mvalid]
            dvcol[base_col[ci_all[mvalid], j] + p_all[mvalid]] = dinv[gids[rr]]

        # slot source table: [4 k-groups, CS cols], -1 = pad (zeros)
        slotsrc = np.full((4, CS), -1, np.int64)
        # self slots (k = 0 -> group 0, sub-slot 0)
        slotsrc[0, base_col[ci_all, 0] + p_all] = gids
        # edge slots (k = 1 + within-count)
        rk = invperm[d_c]
        eorder = np.argsort(rk, kind="stable")
        rk_s = rk[eorder]
        s_s = s_c[eorder]
        starts = np.zeros(NODES_PER_CORE + 1, np.int64)
        np.cumsum(cnt[perm], out=starts[1:])
        within = np.arange(len(rk_s)) - starts[rk_s]
        k = within + 1
        cole = base_col[rk_s // P, k // 4] + (rk_s % P)
        slotsrc[k % 4, cole] = s_s

        tabS = np.zeros((P, CS), bf)
        for g in range(4):
            vals = np.zeros((CS, DIN), np.float32)
            mm = slotsrc[g] >= 0
            vals[mm] = xs_pre[slotsrc[g][mm]]
            vals *= dvcol[:, None]
            tabS[32 * g:32 * g + 32, :] = vals.T.astype(bf)

        perms.append(perm)
        in_maps.append({
            "tabS": tabS,
            "cb": np.concatenate([azS, ahS, Wout], axis=1),
            "cf": np.stack([-az, ah, bb128[:, 0]], axis=1).astype(np.float32),
        })

    if os.environ.get("KERNEL_DEBUG") == "1":
        print(f"[kernel] kq={kq.tolist()} CS={CS} order={s_order} "
              f"stream={P * CS * 2 / 1e6:.2f}MB/core")
    key = ("v14", tuple(kq.tolist()))
    if key not in _cache:
        _cache[key] = _build_device_kernel(kq)
    nc = _cache[key]

    from concourse.bass_utils import run_bass_kernel_spmd
    trace = os.environ.get("KERNEL_TRACE") == "1"
    kwargs = {}
    if trace:
        kwargs = {"trace": True, "tmpdir": os.environ.get("KERNEL_TRACE_DIR", "/tmp/kernel_trace")}
    res = run_bass_kernel_spmd(nc, in_maps, list(range(NCORES)), **kwargs)
    global last_result
    last_result = res

    y = np.empty((NA, NP_), np.float32)
    for c in range(NCORES):
        yc = res.results[c]["y"]                                # [8, 6272]
        lo = c * NODES_PER_CORE
        y[lo + perms[c], :] = yc[:, :NODES_PER_CORE].T
    return y


# revision 37
# speedup vs baseline: 1.1372x; 1.0303x over previous
"""Trainium2 8-core kernel for the GConvGRU-style GNN message-passing net.

Reference computation (N=100000 nodes, E=400000 edges, y = out[:50000]):
    deg  = indeg(dst) + 1;  dinv = rsqrt(deg)
    xs   = D^-1/2 (A + I) D^-1/2 x          # [N, 32] normalized aggregation
    cz   = xs @ Wz + bz ; ch = xs @ Wh + bh # (H == 0 for this problem)
    Z    = sigmoid(cz @ Lz_top + Lz_b); H~ = tanh(ch @ Lh_top + Lh_b)
    Hn   = (1 - Z) * H~
    y    = relu(Hn) @ W_out + b_out         # rows [0, 50000)

Only nodes < 50000 reach the output, so only their in-edges matter.

Sharding: 8 cores x 6250 output nodes. The host stages, per core, a
feature-major bf16 "slot stream" in DRAM — one column per (node,
sub-slot), fully pre-normalized (dinv[src]*dinv[dst]*x edge slots,
dinv^2*x self slot), a node's slots dealt round-robin over 4 k-groups
stacked 4x32 on the partition axis. The device does all arithmetic:

  - PE accumulates the slot sum directly from the stream into PSUM
    (per run of equal-depth chunks: k matmuls with start/stop
    accumulation), folding both the 4-group sum (via the 128-deep
    contraction against the 4x-tiled folded gate weights) and the
    sub-slot sum (via PSUM accumulate). No separate collapse pass.
  - ACT applies sigmoid/tanh per 1024-col superblock (PSUM -> SBUF).
  - DVE fuses relu+gating: prr = (ht max 0) * zc  [one STT op], then
    adds b_out while moving y out of PSUM (tensor_scalar_add).
  - Superblocks are processed smallest-stream-first so compute starts
    as soon as the first (smallest) DMA piece lands; stream pieces are
    issued back-to-back on the sync HWDGE queue and pipeline at line
    rate while the PE consumes earlier pieces.
"""
import os
import sys

import numpy as np

for _p in ("/root/.axon_site", "/root/.axon_site/_ro/trn_rl_repo",
           "/root/.axon_site/_ro/pypackages", "/opt/trn_rl_repo"):
    if os.path.isdir(_p) and _p not in sys.path:
        sys.path.append(_p)

N = 100000
E = 400000
DIN = 32
FLT = 128
NP_ = 8
NA = 50000
NCORES = 8
NODES_PER_CORE = NA // NCORES           # 6250
P = 128
NCHUNK = 49                             # chunks of 128 node cols
NCOL = NCHUNK * P                       # 6272 compute cols
SB_CHUNKS = 12                          # chunks per superblock (1536 cols)

_cache = {}


def _split_sync_waits(nc, mybir, limit=1):
    """walrus CoreV3 codegen supports one sync-wait per instruction."""
    cnt = 0
    for fn in nc.m.functions:
        for bb in fn.blocks:
            insts = list(bb.instructions)
            out = []
            changed = False
            for inst in insts:
                si = inst.sync_info
                if si is not None and si.on_wait is not None and len(si.on_wait) > limit:
                    w = list(si.on_wait)
                    upd = list(si.on_update) if si.on_update else []
                    chunks = [w[i:i + limit] for i in range(0, len(w), limit)]
                    for chunk in chunks[:-1]:
                        d = mybir.InstDrain(name=f"I-wsplit{cnt}", ins=[], outs=[])
                        cnt += 1
                        d.engine = inst.engine
                        d.sync_info = mybir.SyncInfo(on_wait=chunk, on_update=[])
                        out.append(d)
                    inst.sync_info = mybir.SyncInfo(on_wait=chunks[-1], on_update=upd)
                    changed = True
                out.append(inst)
            if changed:
                bb.instructions = out


def _plan(kq):
    """Static schedule shared by all cores.

    Superblocks of SB_CHUNKS chunks; within each 4-chunk half, runs of
    equal slot depth k (so every matmul's PSUM out stays inside one
    512-col bank). Superblocks are processed smallest-stream-first.
    Returns (sbs, order, CS) where sbs[s] = (chunk_lo, chunk_hi, runs,
    stream_off, stream_cols) with runs = [(chunk_lo, nchunks, k,
    stream_off_within_sb)], offsets assigned in process order.
    """
    kq = np.asarray(kq)
    bounds = list(range(0, NCHUNK, SB_CHUNKS)) + [NCHUNK]
    raw = []
    for lo, hi in zip(bounds[:-1], bounds[1:]):
        runs = []
        cols = 0
        for hlo in range(lo, hi, 4):
            hhi = min(hlo + 4, hi)
            c = hlo
            while c < hhi:
                k = int(kq[c])
                e = c
                while e < hhi and kq[e] == k:
                    e += 1
                runs.append((c, e - c, k, cols))
                cols += k * (e - c) * P
                c = e
        raw.append((lo, hi, runs, cols))
    # Process order: start with a medium superblock (its piece lands
    # early but still feeds the PE densely), then the heavy ones, then
    # the light k=1 superblocks last so the post-tanh drain is short.
    desc = sorted(range(len(raw)), key=lambda s: (-raw[s][3], s))
    order = desc[2:3] + desc[0:2] + desc[3:]
    sbs = []
    off = 0
    offs = {}
    for s in order:
        offs[s] = off
        off += raw[s][3]
    for s, (lo, hi, runs, cols) in enumerate(raw):
        sbs.append((lo, hi, runs, offs[s], cols))
    return sbs, order, off


def _build_device_kernel(kq):
    import concourse.bacc as bacc
    import concourse.mybir as mybir
    from concourse.tile import TileContext

    sbs, order, CS = _plan(kq)

    nc = bacc.Bacc("TRN2")
    f32 = mybir.dt.float32
    bf16 = mybir.dt.bfloat16

    tabS = nc.declare_dram_parameter("tabS", [P, CS], bf16, isOutput=False)
    # all constants in two DMAs: cb = azS | ahS | wout (bf16),
    # cf = -az | ah | b_out (f32) — six separate const DMAs serialized
    # ~6us on the scalar queue and stalled the first LDWEIGHTS.
    cb = nc.declare_dram_parameter("cb", [P, 2 * FLT + NP_], bf16, isOutput=False)
    cf = nc.declare_dram_parameter("cf", [P, 3], f32, isOutput=False)
    yout = nc.declare_dram_parameter("y", [NP_, NCOL], f32, isOutput=True)

    with TileContext(nc) as tc:
        with (
            tc.tile_pool(name="const", bufs=1) as cp,
            tc.tile_pool(name="st", bufs=1) as sp,
            tc.tile_pool(name="uzh", bufs=2, space="PSUM") as pz,
            tc.tile_pool(name="yp", bufs=2, space="PSUM") as yp,
            tc.tile_pool(name="zc", bufs=2) as zcp,
            tc.tile_pool(name="ht", bufs=2) as htp,
            tc.tile_pool(name="pr", bufs=2) as prp,
        ):
            # constants FIRST on the sync queue: FIFO puts them ahead of
            # the big stream pieces (on the scalar ring they finished
            # ~8.5us in — the SDMA engines round-robin rings at packet
            # granularity, so small transfers drown behind big ones)
            cb_t = cp.tile([P, 2 * FLT + NP_], bf16)
            nc.sync.dma_start(out=cb_t[:], in_=cb[:, :])
            cf_t = cp.tile([P, 3], f32)
            nc.sync.dma_start(out=cf_t[:], in_=cf[:, :])
            azS_t = cb_t[:, 0:FLT]
            ahS_t = cb_t[:, FLT:2 * FLT]
            wout_t = cb_t[:, 2 * FLT:2 * FLT + NP_]
            azn_t = cf_t[:, 0:1]
            ahb_t = cf_t[:, 1:2]
            bout_t = cf_t[:, 2:3]

            # stream pieces, one per superblock, issued in process order
            st_tiles = {}
            for s in order:
                lo, hi, runs, soff, cols = sbs[s]
                st = sp.tile([P, cols], bf16, tag=f"st{s}")
                nc.sync.dma_start(out=st[:], in_=tabS[:, soff:soff + cols])
                st_tiles[s] = st

            y_sb = cp.tile([NP_, NCOL], f32)
            dum = cp.tile([FLT, 1], bf16)

            # preload both ACT function tables during the DMA head
            nc.scalar.activation(
                out=dum[:], in_=azn_t[:, :1],
                func=mybir.ActivationFunctionType.Tanh, bias=ahb_t[:, :1],
                scale=1.0)
            nc.scalar.activation(
                out=dum[:], in_=azn_t[:, :1],
                func=mybir.ActivationFunctionType.Sigmoid, bias=ahb_t[:, :1],
                scale=-1.0)

            # PE warmup: continuous matmuls on a memset tile (no DMA
            # dependency, so they start ~1us in) accumulate the PE
            # activity HAM needs before it grants 2.4 GHz; without this
            # the heavy gate matmuls all run at 1.2.
            wg = cp.tile([P, 256], bf16)
            nc.vector.memset(wg[:], 0)
            for _w in range(16):
                wt = yp.tile([P, 256], f32, tag="yp")
                nc.tensor.matmul(out=wt[:], lhsT=wg[:, :FLT],
                                 rhs=wg[:], start=True, stop=True)

            for si, s in enumerate(order):
                lo, hi, runs, soff, cols = sbs[s]
                st = st_tiles[s]
                wsb = (hi - lo) * P
                sbcol0 = lo * P

                uz = pz.tile([P, wsb], f32, tag="uzh")
                uh = pz.tile([P, wsb], f32, tag="uzh")
                for lhsT, ups in ((azS_t, uz), (ahS_t, uh)):
                    for rlo, rn, rk, roff in runs:
                        w = rn * P
                        nod0 = (rlo - lo) * P
                        for j in range(rk):
                            nc.tensor.matmul(
                                out=ups[:, nod0:nod0 + w], lhsT=lhsT[:],
                                rhs=st[:, roff + j * w:roff + (j + 1) * w],
                                start=(j == 0), stop=(j == rk - 1))

                zc = zcp.tile([FLT, wsb], bf16, tag="zc")
                nc.scalar.activation(
                    out=zc[:], in_=uz[:],
                    func=mybir.ActivationFunctionType.Sigmoid,
                    bias=azn_t[:, :1], scale=-1.0)
                ht = htp.tile([FLT, wsb], bf16, tag="ht")
                nc.scalar.activation(
                    out=ht[:], in_=uh[:],
                    func=mybir.ActivationFunctionType.Tanh,
                    bias=ahb_t[:, :1], scale=1.0)

                # post-tanh chain at 512-block granularity so the drain of
                # the final superblocks pipelines across engines
                tail = si >= len(order) - 2
                prr = prp.tile([FLT, wsb], bf16, tag="pr")
                for h in range(0, wsb, 512):
                    w2 = min(512, wsb - h)
                    # fused relu+gating on DVE: prr = (ht max 0) * zc
                    nc.vector.scalar_tensor_tensor(
                        out=prr[:, h:h + w2], in0=ht[:, h:h + w2], scalar=0.0,
                        in1=zc[:, h:h + w2],
                        op0=mybir.AluOpType.max, op1=mybir.AluOpType.mult)
                    ypt = yp.tile([NP_, w2], f32, tag="yp")
                    nc.tensor.matmul(out=ypt[:], lhsT=wout_t[:],
                                     rhs=prr[:, h:h + w2],
                                     start=True, stop=True)
                    if tail or (si == len(order) - 3 and h == 1024):
                        # ACT is idle after its last tanh; DVE still owns
                        # the STTs — split the drain across both
                        nc.scalar.activation(
                            out=y_sb[:, sbcol0 + h:sbcol0 + h + w2],
                            in_=ypt[:],
                            func=mybir.ActivationFunctionType.Identity,
                            bias=bout_t[:NP_, :1], scale=1.0)
                    else:
                        nc.vector.tensor_scalar_add(
                            out=y_sb[:, sbcol0 + h:sbcol0 + h + w2], in0=ypt[:],
                            scalar1=bout_t[:NP_, :1])
                    if tail:
                        deng = nc.sync if (h // 512) % 2 == 0 else nc.scalar
                        deng.dma_start(
                            out=yout[:, sbcol0 + h:sbcol0 + h + w2],
                            in_=y_sb[:, sbcol0 + h:sbcol0 + h + w2])
                if not tail:
                    nc.gpsimd.dma_start(out=yout[:, sbcol0:sbcol0 + wsb],
                                        in_=y_sb[:, sbcol0:sbcol0 + wsb])

    import concourse.mybir as mybir2
    _split_sync_waits(nc, mybir2)
    nc.compile()
    return nc


def _numpy_fallback(x, H, edge_index, Wz, bz, Wr, br, Wh, bh,
                    Lz_w, Lz_b, Lr_w, Lr_b, Lh_w, Lh_b, W_out, b_out):
    """Exact replica of the reference for unexpected inputs (H != 0)."""
    src = np.asarray(edge_index[0], dtype=np.int64)
    dst = np.asarray(edge_index[1], dtype=np.int64)
    deg = np.zeros(N, np.float32)
    np.add.at(deg, dst, 1.0)
    deg += 1.0
    dinv = (1.0 / np.sqrt(deg)).astype(np.float32)

    def gcn(W, b):
        h = x @ W
        norm = (dinv[src] * dinv[dst]).astype(np.float32)
        agg = np.zeros_like(h)
        np.add.at(agg, dst, h[src] * norm[:, None])
        agg = agg + h * (dinv * dinv)[:, None]
        return agg + b

    def sigmoid(v):
        return 1.0 / (1.0 + np.exp(-v))

    cz = gcn(Wz, bz)
    cr = gcn(Wr, br)
    ch = gcn(Wh, bh)
    Z = sigmoid(np.concatenate([cz, H], axis=1) @ Lz_w + Lz_b)
    R = sigmoid(np.concatenate([cr, H], axis=1) @ Lr_w + Lr_b)
    Ht = np.tanh(np.concatenate([ch, H * R], axis=1) @ Lh_w + Lh_b)
    Hn = Z * H + (1.0 - Z) * Ht
    y = np.maximum(Hn, 0.0) @ W_out + b_out
    return y[:NA].astype(np.float32)


def kernel(x, H, edge_index, Wz, bz, Wr, br, Wh, bh,
           Lz_w, Lz_b, Lr_w, Lr_b, Lh_w, Lh_b, W_out, b_out):
    x = np.asarray(x, dtype=np.float32)
    H = np.asarray(H)
    if H.size and np.any(H):
        return _numpy_fallback(x, np.asarray(H, np.float32), edge_index,
                               np.asarray(Wz, np.float32), np.asarray(bz, np.float32),
                               np.asarray(Wr, np.float32), np.asarray(br, np.float32),
                               np.asarray(Wh, np.float32), np.asarray(bh, np.float32),
                               np.asarray(Lz_w, np.float32), np.asarray(Lz_b, np.float32),
                               np.asarray(Lr_w, np.float32), np.asarray(Lr_b, np.float32),
                               np.asarray(Lh_w, np.float32), np.asarray(Lh_b, np.float32),
                               np.asarray(W_out, np.float32), np.asarray(b_out, np.float32))

    import ml_dtypes
    bf = ml_dtypes.bfloat16

    src = np.asarray(edge_index[0], dtype=np.int64)
    dst = np.asarray(edge_index[1], dtype=np.int64)

    # --- normalization ---
    deg = np.bincount(dst, minlength=N).astype(np.float32) + 1.0
    dinv = (1.0 / np.sqrt(deg)).astype(np.float32)
    xs_pre = x * dinv[:, None]                                  # dinv[s] * x[s]

    # --- folded gate weights (H = 0 path) ---
    Wz = np.asarray(Wz, np.float32); Wh = np.asarray(Wh, np.float32)
    Lz_top = np.asarray(Lz_w, np.float32)[:FLT]
    Lh_top = np.asarray(Lh_w, np.float32)[:FLT]
    Az = Wz @ Lz_top                                            # [32,128]
    Ah = Wh @ Lh_top
    az = (np.asarray(bz, np.float32) @ Lz_top + np.asarray(Lz_b, np.float32)).astype(np.float32)
    ah = (np.asarray(bh, np.float32) @ Lh_top + np.asarray(Lh_b, np.float32)).astype(np.float32)
    Wout = np.asarray(W_out, np.float32).astype(bf)             # [128,8]
    bout = np.asarray(b_out, np.float32)                        # [8]
    # y-block h of a superblock lands at partitions 32h..32h+7
    bb128 = np.zeros((P, 1), np.float32)
    for h in range(4):
        bb128[32 * h:32 * h + NP_, 0] = bout

    # --- live edges: only dst < NA contribute to the output ---
    live = dst < NA
    srcL = src[live]
    dstL = dst[live]

    # per-core degree-sorted packing; uniform slot profile across cores
    per_core = []
    counts_sorted_all = np.zeros((NCORES, NCOL), np.int64)
    for c in range(NCORES):
        lo, hi = c * NODES_PER_CORE, (c + 1) * NODES_PER_CORE
        m = (dstL >= lo) & (dstL < hi)
        s_c = srcL[m]
        d_c = dstL[m] - lo
        cnt = np.bincount(d_c, minlength=NODES_PER_CORE)
        perm = np.argsort(-cnt, kind="stable")
        counts_sorted_all[c, :NODES_PER_CORE] = cnt[perm]
        per_core.append((s_c, d_c, cnt, perm))

    # per-chunk sub-slot depth (incl. self slot), shared by all cores
    kq = np.zeros(NCHUNK, np.int64)
    for ci in range(NCHUNK):
        kp = counts_sorted_all[:, ci * P:(ci + 1) * P].max() + 1
        kq[ci] = (kp + 3) // 4
    KMAX = int(kq.max())

    sbs, s_order, CS = _plan(kq)
    # base stream col of (chunk ci, sub-slot j), -1 = unused
    base_col = np.full((NCHUNK, KMAX), -1, np.int64)
    for lo, hi, runs, soff, cols in sbs:
        for rlo, rn, rk, roff in runs:
            w = rn * P
            for j in range(rk):
                for ci in range(rlo, rlo + rn):
                    base_col[ci, j] = soff + roff + j * w + (ci - rlo) * P

    in_maps = []
    perms = []
    azS = np.tile(Az, (4, 1)).astype(bf)                        # [128,128]
    ahS = np.tile(Ah, (4, 1)).astype(bf)
    r_all = np.arange(NODES_PER_CORE)
    ci_all = r_all // P
    p_all = r_all % P
    for c in range(NCORES):
        s_c, d_c, cnt, perm = per_core[c]
        invperm = np.empty(NODES_PER_CORE, np.int64)
        invperm[perm] = np.arange(NODES_PER_CORE)
        gids = perm + c * NODES_PER_CORE                        # rank -> node id

        # dinv[dst] per stream column (same for all k-groups)
        dvcol = np.zeros(CS, np.float32)
        for j in range(KMAX):
            mvalid = base_col[ci_all, j] >= 0
            rr = r_all[mvalid]
            dvcol[base_col[ci_all[mvalid], j] + p_all[mvalid]] = dinv[gids[rr]]

        # slot source table: [4 k-groups, CS cols], -1 = pad (zeros)
        slotsrc = np.full((4, CS), -1, np.int64)
        # self slots (k = 0 -> group 0, sub-slot 0)
        slotsrc[0, base_col[ci_all, 0] + p_all] = gids
        # edge slots (k = 1 + within-count)
        rk = invperm[d_c]
        eorder = np.argsort(rk, kind="stable")
        rk_s = rk[eorder]
        s_s = s_c[eorder]
        starts = np.zeros(NODES_PER_CORE + 1, np.int64)
        np.cumsum(cnt[perm], out=starts[1:])
        within = np.arange(len(rk_s)) - starts[rk_s]
        k = within + 1
        cole = base_col[rk_s // P, k // 4] + (rk_s % P)
        slotsrc[k % 4, cole] = s_s

        tabS = np.zeros((P, CS), bf)
        for g in range(4):
            vals = np.zeros((CS, DIN), np.float32)
            mm = slotsrc[g] >= 0
            vals[mm] = xs_pre[slotsrc[g][mm]]
            vals *= dvcol[:, None]
            tabS[32 * g:32 * g + 32, :] = vals.T.astype(bf)

        perms.append(perm)
        in_maps.append({
            "tabS": tabS,
            "cb": np.concatenate([azS, ahS, Wout], axis=1),
            "cf": np.stack([-az, ah, bb128[:, 0]], axis=1).astype(np.float32),
        })

    if os.environ.get("KERNEL_DEBUG") == "1":
        print(f"[kernel] kq={kq.tolist()} CS={CS} order={s_order} "
              f"stream={P * CS * 2 / 1e6:.2f}MB/core")
    key = ("v15", tuple(kq.tolist()))
    if key not in _cache:
        _cache[key] = _build_device_kernel(kq)
    nc = _cache[key]

    from concourse.bass_utils import run_bass_kernel_spmd
    trace = os.environ.get("KERNEL_TRACE") == "1"
    kwargs = {}
    if trace:
        kwargs = {"trace": True, "tmpdir": os.environ.get("KERNEL_TRACE_DIR", "/tmp/kernel_trace")}
    res = run_bass_kernel_spmd(nc, in_maps, list(range(NCORES)), **kwargs)
    global last_result
    last_result = res

    y = np.empty((NA, NP_), np.float32)
    for c in range(NCORES):
        yc = res.results[c]["y"]                                # [8, 6272]
        lo = c * NODES_PER_CORE
        y[lo + perms[c], :] = yc[:, :NODES_PER_CORE].T
    return y
